# revision 2
# baseline (speedup 1.0000x reference)
"""Distributed k-NN (k-th nearest-neighbor distance) on 8 Trainium2 cores.

Strategy: shard x_ref (M=100000) across 8 cores (12500 each). Each core
computes, for every query q and its shard refs r, the score
    s(q, r) = 2*x_q . r - ||r||^2  =  ||x_q||^2 - dist(q, r)^2
via a single K=66 augmented bf16 matmul (stationary = [2*X^T; -1; -1],
moving = [ref^T; r2_hi; r2_lo]); larger s == smaller distance.
Accumulation is fp32 in PSUM.

Drain design (the bottleneck): PSUM can only be read by the DVE
(0.96 GHz, 1 elem/cycle/lane) and the Scalar engine (1.2 GHz,
1 elem/cycle/lane), so the 12544 scores/lane/tile are split:
  - D-side (c0 + half of c1 + 256 tail): DVE max8 straight from PSUM.
  - A-side (rest, 9216/lane): Scalar copies PSUM -> SBUF bf16; the DVE
    then folds pairs with tensor_max (bf16 2x mode, 2 elem/cycle) three
    levels down to 1152/lane and max8's that once.
Both engines run ~9.4us/tile, fully overlapped with the matmuls.

The host merges the 8 cores' 32 candidates/tile and takes the k-th
smallest distance: d_k = sqrt(max(x2 - s_(k), 0)).

Exactness: per-region top-8 capture misses a core's top-k only if a
single region holds >=9 of the query's global top-10 (negligible), or
if two of the top-10 collide in one 8-wide fold group on the A-side
(P ~ 0.3%/query; cost is d_11 instead of d_10, ~5e-3 relative).
"""

import ml_dtypes
import numpy as np

import concourse.mybir as mybir
from concourse import bacc
from concourse.bass_utils import run_bass_kernel_spmd
from concourse.tile import TileContext

P = 128            # queries per tile (SBUF partitions)
NQ = 2048          # total queries
QT = NQ // P       # 16 query tiles
D = 64             # feature dim
KC = D + 2         # contraction dim: 64 coords + r2_hi + r2_lo rows
N_CORES = 8
M_TOTAL = 100000
M_SHARD = M_TOTAL // N_CORES   # 12500
MM_N = 512                     # refs per matmul (1 PSUM bank)
W = 12544                      # padded shard width (6x2048 chunks + 256)
CW = 2048                      # PSUM chunk width (4 banks)
NCAND = 32                     # candidate slots per (query, qtile)
A_W = 9216                     # A-side width: c1[1024:] + c2..c5
NEG = -1.0e30
BF16 = ml_dtypes.bfloat16


def build_kernel(qt: int = QT, w: int = W):
    assert w == 6 * CW + 256
    nc = bacc.Bacc("TRN2", target_bir_lowering=False, debug=False)
    mov_d = nc.dram_tensor(
        "mov", [KC, w], mybir.dt.bfloat16, kind="ExternalInput"
    )
    sta_d = nc.dram_tensor(
        "sta", [qt, KC, P], mybir.dt.bfloat16, kind="ExternalInput"
    )
    out_d = nc.dram_tensor(
        "out", [P, qt * NCAND], mybir.dt.float32, kind="ExternalOutput"
    )
    with TileContext(nc) as tc:
        with (
            tc.tile_pool(name="mov_pool", bufs=1) as mov_pool,
            tc.tile_pool(name="sta_pool", bufs=1) as sta_pool,
            tc.tile_pool(name="cp_pool", bufs=2) as cp_pool,
            tc.tile_pool(name="fold_pool", bufs=2) as fold_pool,
            tc.tile_pool(name="out_pool", bufs=1) as out_pool,
            tc.tile_pool(name="psum", bufs=2, space="PSUM") as psum_pool,
        ):
            # chunk layout: 6 x 2048 + 256 tail
            chunks = [CW] * 6 + [256]
            mov_tiles = []
            off = 0
            for c, cw in enumerate(chunks):
                t = mov_pool.tile([KC, cw], mybir.dt.bfloat16, tag=f"mov{c}")
                nc.sync.dma_start(t, mov_d[:, off:off + cw])
                mov_tiles.append(t)
                off += cw
            sta_tiles = []
            for t_ in range(qt):
                s = sta_pool.tile([KC, P], mybir.dt.bfloat16, tag=f"sta{t_}")
                nc.sync.dma_start(s, sta_d[t_])
                sta_tiles.append(s)
            out_buf = out_pool.tile([P, qt * NCAND], mybir.dt.float32)
            for t_ in range(qt):
                cp = cp_pool.tile([P, A_W], mybir.dt.bfloat16, tag="cp")
                f1 = fold_pool.tile([P, A_W // 2], mybir.dt.bfloat16, tag="f1")
                f2 = fold_pool.tile([P, A_W // 4], mybir.dt.bfloat16, tag="f2")
                f3 = fold_pool.tile([P, A_W // 8], mybir.dt.bfloat16, tag="f3")
                ob = t_ * NCAND
                ps = []  # psum tile per chunk (ping-pong via pool bufs=2)
                for c, cw in enumerate(chunks):
                    pp = psum_pool.tile([P, CW], mybir.dt.float32, tag="ps")
                    for h in range(0, cw, MM_N):
                        hw = min(MM_N, cw - h)
                        nc.tensor.matmul(
                            pp[:, h:h + hw],
                            lhsT=sta_tiles[t_],
                            rhs=mov_tiles[c][:, h:h + hw],
                            start=True, stop=True,
                        )
                    ps.append(pp)
                    # consumers, issued right after each chunk's matmuls
                    if c == 0:
                        nc.vector.max(out=out_buf[:, ob:ob + 8],
                                      in_=pp[:, 0:CW])
                    elif c == 1:
                        nc.vector.max(out=out_buf[:, ob + 8:ob + 16],
                                      in_=pp[:, 0:1024])
                        nc.scalar.activation(
                            out=cp[:, 0:1024], in_=pp[:, 1024:2048],
                            func=mybir.ActivationFunctionType.Copy,
                        )
                    elif c < 6:
                        doff = 1024 + (c - 2) * CW
                        nc.scalar.activation(
                            out=cp[:, doff:doff + CW], in_=pp[:, 0:CW],
                            func=mybir.ActivationFunctionType.Copy,
                        )
                    else:  # 256 tail
                        nc.vector.max(out=out_buf[:, ob + 16:ob + 24],
                                      in_=pp[:, 0:256])
                # fold tree on the A-side copies (bf16, 2x DVE mode)
                h = A_W // 2
                nc.vector.tensor_max(f1, cp[:, 0:h], cp[:, h:A_W])
                nc.vector.tensor_max(f2, f1[:, 0:h // 2], f1[:, h // 2:h])
                nc.vector.tensor_max(f3, f2[:, 0:h // 4], f2[:, h // 4:h // 2])
                nc.vector.max(out=out_buf[:, ob + 24:ob + 32], in_=f3)
            nc.sync.dma_start(out_d[:, :], out_buf)
    nc.compile()
    return nc


def prep_inputs(X: np.ndarray, x_ref: np.ndarray):
    """Host-side shard/layout prep. Returns (in_maps, x2)."""
    X = np.ascontiguousarray(X, dtype=np.float32)
    x_ref = np.ascontiguousarray(x_ref, dtype=np.float32)

    sta = np.empty((QT, KC, P), BF16)
    Xt = X.reshape(QT, P, D)
    sta[:, :D, :] = (2.0 * Xt.transpose(0, 2, 1)).astype(BF16)
    sta[:, D, :] = -1.0
    sta[:, D + 1, :] = -1.0
    x2 = np.sum(X.astype(np.float64) * X, axis=1).astype(np.float32)  # [NQ]

    in_maps = []
    for core in range(N_CORES):
        shard = x_ref[core * M_SHARD:(core + 1) * M_SHARD]      # [12500, 64]
        r2 = np.sum(shard.astype(np.float64) * shard, axis=1).astype(np.float32)
        r2_hi = r2.astype(BF16)
        r2_lo = (r2 - r2_hi.astype(np.float32)).astype(BF16)
        aug = np.zeros((W, KC), BF16)
        aug[:M_SHARD, :D] = shard.astype(BF16)
        aug[:M_SHARD, D] = r2_hi
        aug[:M_SHARD, D + 1] = r2_lo
        aug[M_SHARD:, D] = 1.0e30        # padded cols -> s = -1e30
        mov = np.ascontiguousarray(aug.T)                        # [66, 12544]
        in_maps.append({"mov": mov, "sta": sta})
    return in_maps, x2


_NC_CACHE = {}


def get_nc():
    if "nc" not in _NC_CACHE:
        _NC_CACHE["nc"] = build_kernel()
    return _NC_CACHE["nc"]


def kernel(X: np.ndarray, x_ref: np.ndarray, k) -> np.ndarray:
    k = int(k)
    assert 1 <= k <= 16, f"merge path supports k<=16, got {k}"
    assert X.shape == (NQ, D) and x_ref.shape == (M_TOTAL, D)

    in_maps, x2 = prep_inputs(X, x_ref)
    nc = get_nc()

    res = run_bass_kernel_spmd(nc, in_maps, core_ids=list(range(N_CORES)))
    # [8, P, QT*NCAND] -> per query 8*NCAND candidate scores
    cands = np.stack([r["out"] for r in res.results])
    cands = cands.reshape(N_CORES, P, QT, NCAND)
    # query q = t*P + p  ->  cands[:, p, t, :]
    cands = cands.transpose(2, 1, 0, 3).reshape(NQ, N_CORES * NCAND)
    # k-th largest score s_(k) == k-th smallest distance
    s_k = -np.partition(-cands, k - 1, axis=1)[:, k - 1]
    d = np.sqrt(np.maximum(x2 - s_k, 0.0))
    return d.astype(np.float32)


# revision 8
# speedup vs baseline: 1.0401x; 1.0401x over previous
"""Distributed k-NN (k-th nearest-neighbor distance) on 8 Trainium2 cores.

Each core computes p(q, r) = 2*x_q . r for its 12288-column ref shard via
K=64 bf16 matmuls, packed TWO query tiles per pass with PE row-tiling:
strip A occupies PE rows 0-63, strip B rows 64-127 (tile_position
(0,0)/(64,0)), streaming concurrently -> 2x column throughput, which
keeps the PE off the critical path even when the HAM clock gate holds it
at 1.2 GHz.

The -||r||^2 term is handled by r2-sorting the refs so every on-device
comparison region is r2-homogeneous:
  - 1024 lowest + 1024 highest r2 refs are scored EXACTLY on the host
    (they are where sorted-r2 spacing is wide) and never reach a device
    candidate region.
  - low/high zones are dealt round-robin (rank mod 8) across cores, so a
    norm-concentrated query's top-k spreads over all cores; each core's
    A-side holds them in fold groups of 9 consecutive core-ranks
    (r2 spread ~0.02-0.2), subtracting per-group r2 means on device.
  - the dense middle zone is sliced contiguously per core into the
    D-side regions (r2 spread ~0.3 per 1024), whose candidate values the
    host corrects by region means.

Drain (the bottleneck): PSUM is readable only by DVE (0.96 GHz, 1
elem/cycle/lane) and ACT (1.2 GHz, 1 elem/cycle/lane). Per query tile
(12288 scores/lane): D-side chunks d0-d2 are max8'ed by the DVE straight
from PSUM; A-side chunks a0-a8 (9216/lane) are copied by ACT to SBUF
bf16, where the DVE folds groups of 9 with tensor_max (bf16 2x mode),
subtracts group r2 means, and max8's each half. Host merges
8 cores x 16 tiles x 40 candidates + 2048 exact extreme scores and takes
the k-th: d_k = sqrt(max(x2 - s_(k), 0)).
"""

import ml_dtypes
import numpy as np

import concourse.mybir as mybir
from concourse import bacc
from concourse.bass_utils import run_bass_kernel_spmd
from concourse.tile import TileContext

P = 128            # queries per tile (SBUF partitions)
NQ = 2048          # total queries
QT = NQ // P       # 16 query tiles
NP = QT // 2       # 8 passes, two query tiles per pass
D = 64             # feature dim (= contraction dim)
N_CORES = 8
M_TOTAL = 100000
M_SHARD = M_TOTAL // N_CORES   # 12500
MM_N = 512                     # refs per matmul (1 PSUM bank)
CW = 1024                      # PSUM chunk width (2 banks)
ND = 3                         # D-side 1024-chunks (direct PSUM max8)
NA = 9                         # A-side 1024-chunks (ACT copy + fold)
W = (ND + NA) * CW             # 12288 shard width
A_W = NA * CW                  # 9216
NCAND = 40                     # per (query, qtile): 3x8 D + 2x8 A

# global r2-sorted zoning
N_EXT = 2048                   # 1024 lowest + 1024 highest: host-exact
LOW_N = 36720                  # ranks [1024, 38744): dealt mod 8 (4590/core)
MID_LO = 1024 + LOW_N          # 37744
MID_N = ND * CW * N_CORES      # 24576 middle ranks: contiguous slices
MID_HI = MID_LO + MID_N        # 62320
HIGH_HI = M_TOTAL - 1024       # 98976; high zone [62320, 98976): 4582/core
NPAD = A_W - LOW_N // N_CORES - (HIGH_HI - MID_HI) // N_CORES  # 44

BF16 = ml_dtypes.bfloat16


def build_kernel():
    nc = bacc.Bacc("TRN2", target_bir_lowering=False, debug=False)
    mov_d = nc.dram_tensor(
        "mov", [P, W], mybir.dt.bfloat16, kind="ExternalInput"
    )
    sta_d = nc.dram_tensor(
        "sta", [NP, P, P], mybir.dt.bfloat16, kind="ExternalInput"
    )
    r2g_d = nc.dram_tensor(
        "r2g", [P, CW], mybir.dt.bfloat16, kind="ExternalInput"
    )
    out_d = nc.dram_tensor(
        "out", [P, QT * NCAND], mybir.dt.float32, kind="ExternalOutput"
    )
    NCHUNK = ND + NA
    with TileContext(nc) as tc:
        with (
            tc.tile_pool(name="mov_pool", bufs=1) as mov_pool,
            tc.tile_pool(name="sta_pool", bufs=1) as sta_pool,
            tc.tile_pool(name="cp_pool", bufs=2) as cp_pool,
            tc.tile_pool(name="fold_pool", bufs=1) as fold_pool,
            tc.tile_pool(name="out_pool", bufs=1) as out_pool,
            tc.tile_pool(name="psA", bufs=2, space="PSUM") as psA_pool,
            tc.tile_pool(name="psB", bufs=2, space="PSUM") as psB_pool,
        ):
            mov_tiles = []
            for c in range(NCHUNK):
                t = mov_pool.tile([P, CW], mybir.dt.bfloat16, tag=f"mov{c}")
                nc.sync.dma_start(t, mov_d[:, c * CW:(c + 1) * CW])
                mov_tiles.append(t)
            sta_tiles = []
            for p_ in range(NP):
                s = sta_pool.tile([P, P], mybir.dt.bfloat16, tag=f"sta{p_}")
                nc.sync.dma_start(s, sta_d[p_])
                sta_tiles.append(s)
            r2g = mov_pool.tile([P, CW], mybir.dt.bfloat16, tag="r2g")
            nc.sync.dma_start(r2g, r2g_d[:, :])
            out_buf = out_pool.tile([P, QT * NCAND], mybir.dt.float32)

            for p_ in range(NP):
                # strip A = query tile 2p_, strip B = query tile 2p_+1
                obs = [(2 * p_) * NCAND, (2 * p_ + 1) * NCAND]
                cps = []
                for s in range(2):
                    cp_t = cp_pool.tile([P, A_W], mybir.dt.bfloat16,
                                        tag=f"cp{s}", name=f"cp{s}")
                    cps.append(cp_t)
                for c in range(NCHUNK):
                    pps = [psA_pool.tile([P, CW], mybir.dt.float32,
                                         tag="pa", name="pa"),
                           psB_pool.tile([P, CW], mybir.dt.float32,
                                         tag="pb", name="pb")]
                    for s in range(2):
                        lo = 64 * s
                        for h in range(0, CW, MM_N):
                            nc.tensor.matmul(
                                pps[s][:, h:h + MM_N],
                                lhsT=sta_tiles[p_][lo:lo + 64, :],
                                rhs=mov_tiles[c][lo:lo + 64, h:h + MM_N],
                                start=True, stop=True,
                                tile_position=(lo, 0),
                            )
                    for s in range(2):
                        ob = obs[s]
                        if c < ND:                      # direct PSUM max8
                            nc.vector.max(
                                out=out_buf[:, ob + 8 * c:ob + 8 * c + 8],
                                in_=pps[s][:, 0:CW])
                        else:                           # ACT copy -> SBUF
                            doff = (c - ND) * CW
                            nc.scalar.activation(
                                out=cps[s][:, doff:doff + CW],
                                in_=pps[s][:, 0:CW],
                                func=mybir.ActivationFunctionType.Copy)
                # fold trees (interleave with next chunks via scheduler)
                for s in range(2):
                    cp = cps[s]
                    ob = obs[s]
                    u = fold_pool.tile([P, 4 * CW], mybir.dt.bfloat16,
                                       tag=f"u{s}")
                    v = fold_pool.tile([P, 2 * CW], mybir.dt.bfloat16,
                                       tag=f"v{s}")
                    w = fold_pool.tile([P, CW], mybir.dt.bfloat16,
                                       tag=f"w{s}")
                    z = fold_pool.tile([P, CW], mybir.dt.bfloat16,
                                       tag=f"z{s}")
                    z2 = fold_pool.tile([P, CW], mybir.dt.bfloat16,
                                        tag=f"z2{s}")
                    for i in range(4):  # L1: a(2i) vs a(2i+1)
                        nc.vector.tensor_max(
                            u[:, i * CW:(i + 1) * CW],
                            cp[:, 2 * i * CW:(2 * i + 1) * CW],
                            cp[:, (2 * i + 1) * CW:(2 * i + 2) * CW])
                    for i in range(2):  # L2
                        nc.vector.tensor_max(
                            v[:, i * CW:(i + 1) * CW],
                            u[:, 2 * i * CW:(2 * i + 1) * CW],
                            u[:, (2 * i + 1) * CW:(2 * i + 2) * CW])
                    nc.vector.tensor_max(w, v[:, 0:CW], v[:, CW:2 * CW])
                    nc.vector.tensor_max(z, w, cp[:, 8 * CW:9 * CW])
                    nc.vector.tensor_sub(z2, z, r2g)
                    nc.vector.max(out=out_buf[:, ob + 24:ob + 32],
                                  in_=z2[:, 0:CW // 2])
                    nc.vector.max(out=out_buf[:, ob + 32:ob + 40],
                                  in_=z2[:, CW // 2:CW])
            nc.sync.dma_start(out_d[:, :], out_buf)
    nc.compile()
    return nc


def prep_inputs(X: np.ndarray, x_ref: np.ndarray):
    """Host-side shard/layout prep.

    Returns (in_maps, x2, dadj, ext_idx):
      dadj[core, 3]  D-region r2 means (host candidate correction)
      ext_idx        global indices of the 2048 r2-extreme refs, scored
                     exactly on the host
    """
    X = np.ascontiguousarray(X, dtype=np.float32)
    x_ref = np.ascontiguousarray(x_ref, dtype=np.float32)

    sta = np.empty((NP, P, P), BF16)
    Xt = X.reshape(QT, P, D)
    for p_ in range(NP):
        sta[p_, 0:64, :] = (2.0 * Xt[2 * p_].T).astype(BF16)
        sta[p_, 64:128, :] = (2.0 * Xt[2 * p_ + 1].T).astype(BF16)
    x2 = np.sum(X.astype(np.float64) * X, axis=1).astype(np.float32)

    r2_all = np.sum(x_ref.astype(np.float64) * x_ref, axis=1).astype(np.float32)
    order = np.argsort(r2_all, kind="stable")

    ext_idx = np.concatenate([order[:1024], order[HIGH_HI:]])
    low = order[1024:MID_LO]        # dealt mod 8
    mid = order[MID_LO:MID_HI]      # contiguous 3072-slices
    high = order[MID_HI:HIGH_HI]    # dealt mod 8

    in_maps = []
    dadj_list = []
    for core in range(N_CORES):
        lo_i = low[core::N_CORES]                       # 4590, ascending
        hi_i = high[core::N_CORES]                      # 4582, ascending
        a_i = np.concatenate([lo_i, hi_i, hi_i[-NPAD:]])  # 9216
        d_i = mid[core * ND * CW:(core + 1) * ND * CW]  # 3072, ascending

        # positions: D chunk c=0..2 at [c*CW, (c+1)*CW); A member c of
        # group j at (ND+c)*CW + j, groups = 9 consecutive a-ranks
        pos_a = (ND + np.arange(A_W) % NA) * CW + np.arange(A_W) // NA
        idx_w = np.empty(W, np.int64)
        idx_w[np.arange(ND * CW)] = d_i
        idx_w[pos_a] = a_i

        coords = x_ref[idx_w].astype(BF16)              # [W, 64]
        mov = np.empty((P, W), BF16)
        mov[0:64] = coords.T
        mov[64:128] = coords.T

        ga = r2_all[a_i].reshape(CW, NA).mean(axis=1).astype(BF16)
        r2g = np.broadcast_to(ga[None, :], (P, CW)).copy()

        dadj = r2_all[d_i].reshape(ND, CW).mean(axis=1).astype(np.float32)
        dadj_list.append(dadj)
        in_maps.append({"mov": mov, "sta": sta, "r2g": r2g})
    return in_maps, x2, np.stack(dadj_list), ext_idx


_NC_CACHE = {}


def get_nc():
    if "nc" not in _NC_CACHE:
        _NC_CACHE["nc"] = build_kernel()
    return _NC_CACHE["nc"]


def kernel(X: np.ndarray, x_ref: np.ndarray, k) -> np.ndarray:
    k = int(k)
    assert 1 <= k <= 16, f"merge path supports k<=16, got {k}"
    assert X.shape == (NQ, D) and x_ref.shape == (M_TOTAL, D)

    in_maps, x2, dadj, ext_idx = prep_inputs(X, x_ref)
    nc = get_nc()

    res = run_bass_kernel_spmd(nc, in_maps, core_ids=list(range(N_CORES)))
    cands = np.stack([r["out"] for r in res.results])   # [8, P, QT*NCAND]
    cands = cands.reshape(N_CORES, P, QT, NCAND)
    # host r2 corrections for the D-side slots (0:24); A-side was
    # corrected on device.
    corr = np.zeros((N_CORES, 1, 1, NCAND), np.float32)
    corr[:, 0, 0, 0:24] = np.repeat(dadj, 8, axis=1)
    cands = cands - corr
    # query q = t*P + p  ->  cands[:, p, t, :]
    cands = cands.transpose(2, 1, 0, 3).reshape(NQ, N_CORES * NCAND)

    # exact host scores for the 2048 r2-extreme refs
    xe = np.ascontiguousarray(x_ref[ext_idx], dtype=np.float32)
    s_ext = 2.0 * np.ascontiguousarray(X, np.float32) @ xe.T \
        - np.sum(xe * xe, axis=1)[None, :]
    cands = np.concatenate([cands, s_ext.astype(np.float32)], axis=1)

    s_k = -np.partition(-cands, k - 1, axis=1)[:, k - 1]
    d = np.sqrt(np.maximum(x2 - s_k, 0.0))
    return d.astype(np.float32)


# revision 10
# speedup vs baseline: 1.3570x; 1.3046x over previous
"""Distributed k-NN (k-th nearest-neighbor distance) on 8 Trainium2 cores.

Each core computes p(q, r) = 2*x_q . r for its 12288-column ref shard via
K=64 bf16 matmuls, packed TWO query tiles per pass with PE row-tiling:
strip A occupies PE rows 0-63, strip B rows 64-127 (tile_position
(0,0)/(64,0)), streaming concurrently -> 2x column throughput, which
keeps the PE off the critical path even when the HAM clock gate holds it
at 1.2 GHz.

The -||r||^2 term is handled by r2-sorting the refs so every on-device
comparison region is r2-homogeneous:
  - 1024 lowest + 1024 highest r2 refs are scored EXACTLY on the host
    (they are where sorted-r2 spacing is wide) and never reach a device
    candidate region.
  - low/high zones are dealt round-robin (rank mod 8) across cores, so a
    norm-concentrated query's top-k spreads over all cores; each core's
    A-side holds them in fold groups of 9 consecutive core-ranks
    (r2 spread ~0.02-0.2), subtracting per-group r2 means on device.
  - the dense middle zone is sliced contiguously per core into the
    D-side regions (r2 spread ~0.3 per 1024), whose candidate values the
    host corrects by region means.

Drain (the bottleneck): PSUM is readable only by DVE (0.96 GHz, 1
elem/cycle/lane) and ACT (1.2 GHz, 1 elem/cycle/lane). Per query tile
(12288 scores/lane): D-side chunks d0-d2 are max8'ed by the DVE straight
from PSUM; A-side chunks a0-a8 (9216/lane) are copied by ACT to SBUF
bf16, where the DVE folds groups of 9 with tensor_max (bf16 2x mode),
subtracts group r2 means, and max8's each half. Host merges
8 cores x 16 tiles x 40 candidates + 2048 exact extreme scores and takes
the k-th: d_k = sqrt(max(x2 - s_(k), 0)).
"""

import ml_dtypes
import numpy as np

import concourse.mybir as mybir
from concourse import bacc
from concourse.bass_utils import run_bass_kernel_spmd
from concourse.tile import TileContext

P = 128            # queries per tile (SBUF partitions)
NQ = 2048          # total queries
QT = NQ // P       # 16 query tiles
NP = QT // 2       # 8 passes, two query tiles per pass
D = 64             # feature dim (= contraction dim)
N_CORES = 8
M_TOTAL = 100000
M_SHARD = M_TOTAL // N_CORES   # 12500
MM_N = 512                     # refs per matmul (1 PSUM bank)
CW = 1024                      # PSUM chunk width (2 banks)
ND = 3                         # D-side 1024-chunks (direct PSUM max8)
NA = 9                         # A-side 1024-chunks (ACT copy + fold)
W = (ND + NA) * CW             # 12288 shard width
A_W = NA * CW                  # 9216
NCAND = 40                     # per (query, qtile): 3x8 D + 2x8 A

# global r2-sorted zoning
N_EXT = 2048                   # 1024 lowest + 1024 highest: host-exact
LOW_N = 36720                  # ranks [1024, 38744): dealt mod 8 (4590/core)
MID_LO = 1024 + LOW_N          # 37744
MID_N = ND * CW * N_CORES      # 24576 middle ranks: contiguous slices
MID_HI = MID_LO + MID_N        # 62320
HIGH_HI = M_TOTAL - 1024       # 98976; high zone [62320, 98976): 4582/core
NPAD = A_W - LOW_N // N_CORES - (HIGH_HI - MID_HI) // N_CORES  # 44

BF16 = ml_dtypes.bfloat16


def build_kernel():
    nc = bacc.Bacc("TRN2", target_bir_lowering=False, debug=False)
    mov_d = nc.dram_tensor(
        "mov", [P, W], mybir.dt.bfloat16, kind="ExternalInput"
    )
    sta_d = nc.dram_tensor(
        "sta", [NP, P, P], mybir.dt.bfloat16, kind="ExternalInput"
    )
    r2g_d = nc.dram_tensor(
        "r2g", [P, CW], mybir.dt.bfloat16, kind="ExternalInput"
    )
    out_d = nc.dram_tensor(
        "out", [P, QT * NCAND], mybir.dt.float32, kind="ExternalOutput"
    )
    NCHUNK = ND + NA
    with TileContext(nc) as tc:
        with (
            tc.tile_pool(name="mov_pool", bufs=1) as mov_pool,
            tc.tile_pool(name="sta_pool", bufs=1) as sta_pool,
            tc.tile_pool(name="cp_pool", bufs=2) as cp_pool,
            tc.tile_pool(name="fold_pool", bufs=1) as fold_pool,
            tc.tile_pool(name="out_pool", bufs=1) as out_pool,
            # one single-buffer PSUM pool per (strip, consumer type) so the
            # ACT copy chain and the DVE max8 chain pace independently
            tc.tile_pool(name="psAa", bufs=1, space="PSUM") as psAa_pool,
            tc.tile_pool(name="psAb", bufs=1, space="PSUM") as psAb_pool,
            tc.tile_pool(name="psDa", bufs=1, space="PSUM") as psDa_pool,
            tc.tile_pool(name="psDb", bufs=1, space="PSUM") as psDb_pool,
        ):
            a_pools = [psAa_pool, psAb_pool]
            d_pools = [psDa_pool, psDb_pool]
            mov_tiles = []
            for c in range(NCHUNK):
                t = mov_pool.tile([P, CW], mybir.dt.bfloat16, tag=f"mov{c}")
                nc.sync.dma_start(t, mov_d[:, c * CW:(c + 1) * CW])
                mov_tiles.append(t)
            sta_tiles = []
            for p_ in range(NP):
                s = sta_pool.tile([P, P], mybir.dt.bfloat16, tag=f"sta{p_}")
                nc.sync.dma_start(s, sta_d[p_])
                sta_tiles.append(s)
            r2g = mov_pool.tile([P, CW], mybir.dt.bfloat16, tag="r2g")
            nc.sync.dma_start(r2g, r2g_d[:, :])
            out_buf = out_pool.tile([P, QT * NCAND], mybir.dt.float32)

            def mm_pair(p_, c, pps):
                """Issue the concurrent 2-strip matmul pair for chunk c."""
                for h in range(0, CW, MM_N):
                    for s in range(2):
                        lo = 64 * s
                        nc.tensor.matmul(
                            pps[s][:, h:h + MM_N],
                            lhsT=sta_tiles[p_][lo:lo + 64, :],
                            rhs=mov_tiles[c][lo:lo + 64, h:h + MM_N],
                            start=True, stop=True,
                            tile_position=(lo, 0),
                        )

            def d_block(p_):
                """D-chunk matmuls + max8s for pass p_ (deferred one pass)."""
                for i in range(ND):
                    pps = [d_pools[s].tile([P, CW], mybir.dt.float32,
                                           tag="pd", name="pd")
                           for s in range(2)]
                    mm_pair(p_, NA + i, pps)
                    for s in range(2):
                        ob = (2 * p_ + s) * NCAND
                        nc.vector.max(
                            out=out_buf[:, ob + 8 * i:ob + 8 * i + 8],
                            in_=pps[s][:, 0:CW])

            for p_ in range(NP):
                # strip A = query tile 2p_, strip B = query tile 2p_+1
                obs = [(2 * p_) * NCAND, (2 * p_ + 1) * NCAND]
                cps = []
                for s in range(2):
                    cp_t = cp_pool.tile([P, A_W], mybir.dt.bfloat16,
                                        tag=f"cp{s}", name=f"cp{s}")
                    cps.append(cp_t)
                us, vs, ws, zs, z2s = [], [], [], [], []
                for s in range(2):
                    us.append(fold_pool.tile([P, 4 * CW], mybir.dt.bfloat16,
                                             tag=f"u{s}", name=f"u{s}"))
                    vs.append(fold_pool.tile([P, 2 * CW], mybir.dt.bfloat16,
                                             tag=f"v{s}", name=f"v{s}"))
                    ws.append(fold_pool.tile([P, CW], mybir.dt.bfloat16,
                                             tag=f"w{s}", name=f"w{s}"))
                    zs.append(fold_pool.tile([P, CW], mybir.dt.bfloat16,
                                             tag=f"z{s}", name=f"z{s}"))
                    z2s.append(fold_pool.tile([P, CW], mybir.dt.bfloat16,
                                              tag=f"z2{s}", name=f"z2{s}"))
                for c in range(NA):
                    pps = [a_pools[s].tile([P, CW], mybir.dt.float32,
                                           tag="pa", name="pa")
                           for s in range(2)]
                    mm_pair(p_, c, pps)
                    for s in range(2):
                        nc.scalar.activation(
                            out=cps[s][:, c * CW:(c + 1) * CW],
                            in_=pps[s][:, 0:CW],
                            func=mybir.ActivationFunctionType.Copy)
                    # interleave fold levels as their inputs complete
                    if c in (1, 3, 5, 7):
                        i = c // 2
                        for s in range(2):
                            nc.vector.tensor_max(
                                us[s][:, i * CW:(i + 1) * CW],
                                cps[s][:, (c - 1) * CW:c * CW],
                                cps[s][:, c * CW:(c + 1) * CW])
                        if c in (3, 7):
                            i = c // 4
                            for s in range(2):
                                nc.vector.tensor_max(
                                    vs[s][:, i * CW:(i + 1) * CW],
                                    us[s][:, 2 * i * CW:(2 * i + 1) * CW],
                                    us[s][:, (2 * i + 1) * CW:(2 * i + 2) * CW])
                        if c == 7:
                            for s in range(2):
                                nc.vector.tensor_max(
                                    ws[s], vs[s][:, 0:CW], vs[s][:, CW:2 * CW])
                # tail of the fold tree + deferred D-block of previous pass
                for s in range(2):
                    nc.vector.tensor_max(zs[s], ws[s],
                                         cps[s][:, 8 * CW:9 * CW])
                    nc.vector.tensor_sub(z2s[s], zs[s], r2g)
                    nc.vector.max(out=out_buf[:, obs[s] + 24:obs[s] + 32],
                                  in_=z2s[s][:, 0:CW // 2])
                    nc.vector.max(out=out_buf[:, obs[s] + 32:obs[s] + 40],
                                  in_=z2s[s][:, CW // 2:CW])
                if p_ > 0:
                    d_block(p_ - 1)
            d_block(NP - 1)
            nc.sync.dma_start(out_d[:, :], out_buf)
    nc.compile()
    return nc


def prep_inputs(X: np.ndarray, x_ref: np.ndarray):
    """Host-side shard/layout prep.

    Returns (in_maps, x2, dadj, ext_idx):
      dadj[core, 3]  D-region r2 means (host candidate correction)
      ext_idx        global indices of the 2048 r2-extreme refs, scored
                     exactly on the host
    """
    X = np.ascontiguousarray(X, dtype=np.float32)
    x_ref = np.ascontiguousarray(x_ref, dtype=np.float32)

    sta = np.empty((NP, P, P), BF16)
    Xt = X.reshape(QT, P, D)
    for p_ in range(NP):
        sta[p_, 0:64, :] = (2.0 * Xt[2 * p_].T).astype(BF16)
        sta[p_, 64:128, :] = (2.0 * Xt[2 * p_ + 1].T).astype(BF16)
    x2 = np.sum(X.astype(np.float64) * X, axis=1).astype(np.float32)

    r2_all = np.sum(x_ref.astype(np.float64) * x_ref, axis=1).astype(np.float32)
    order = np.argsort(r2_all, kind="stable")

    ext_idx = np.concatenate([order[:1024], order[HIGH_HI:]])
    low = order[1024:MID_LO]        # dealt mod 8
    mid = order[MID_LO:MID_HI]      # contiguous 3072-slices
    high = order[MID_HI:HIGH_HI]    # dealt mod 8

    in_maps = []
    dadj_list = []
    for core in range(N_CORES):
        lo_i = low[core::N_CORES]                       # 4590, ascending
        hi_i = high[core::N_CORES]                      # 4582, ascending
        a_i = np.concatenate([lo_i, hi_i, hi_i[-NPAD:]])  # 9216
        d_i = mid[core * ND * CW:(core + 1) * ND * CW]  # 3072, ascending

        # positions: A member m of group j at m*CW + j (chunks 0..8,
        # groups = 9 consecutive a-ranks); D chunks at 9*CW + [0, 3*CW)
        pos_a = (np.arange(A_W) % NA) * CW + np.arange(A_W) // NA
        idx_w = np.empty(W, np.int64)
        idx_w[NA * CW + np.arange(ND * CW)] = d_i
        idx_w[pos_a] = a_i

        coords = x_ref[idx_w].astype(BF16)              # [W, 64]
        mov = np.empty((P, W), BF16)
        mov[0:64] = coords.T
        mov[64:128] = coords.T

        ga = r2_all[a_i].reshape(CW, NA).mean(axis=1).astype(BF16)
        r2g = np.broadcast_to(ga[None, :], (P, CW)).copy()

        dadj = r2_all[d_i].reshape(ND, CW).mean(axis=1).astype(np.float32)
        dadj_list.append(dadj)
        in_maps.append({"mov": mov, "sta": sta, "r2g": r2g})
    return in_maps, x2, np.stack(dadj_list), ext_idx


_NC_CACHE = {}


def get_nc():
    if "nc" not in _NC_CACHE:
        _NC_CACHE["nc"] = build_kernel()
    return _NC_CACHE["nc"]


def kernel(X: np.ndarray, x_ref: np.ndarray, k) -> np.ndarray:
    k = int(k)
    assert 1 <= k <= 16, f"merge path supports k<=16, got {k}"
    assert X.shape == (NQ, D) and x_ref.shape == (M_TOTAL, D)

    in_maps, x2, dadj, ext_idx = prep_inputs(X, x_ref)
    nc = get_nc()

    res = run_bass_kernel_spmd(nc, in_maps, core_ids=list(range(N_CORES)))
    cands = np.stack([r["out"] for r in res.results])   # [8, P, QT*NCAND]
    cands = cands.reshape(N_CORES, P, QT, NCAND)
    # host r2 corrections for the D-side slots (0:24); A-side was
    # corrected on device.
    corr = np.zeros((N_CORES, 1, 1, NCAND), np.float32)
    corr[:, 0, 0, 0:24] = np.repeat(dadj, 8, axis=1)
    cands = cands - corr
    # query q = t*P + p  ->  cands[:, p, t, :]
    cands = cands.transpose(2, 1, 0, 3).reshape(NQ, N_CORES * NCAND)

    # exact host scores for the 2048 r2-extreme refs
    xe = np.ascontiguousarray(x_ref[ext_idx], dtype=np.float32)
    s_ext = 2.0 * np.ascontiguousarray(X, np.float32) @ xe.T \
        - np.sum(xe * xe, axis=1)[None, :]
    cands = np.concatenate([cands, s_ext.astype(np.float32)], axis=1)

    s_k = -np.partition(-cands, k - 1, axis=1)[:, k - 1]
    d = np.sqrt(np.maximum(x2 - s_k, 0.0))
    return d.astype(np.float32)
